# revision 14
# baseline (speedup 1.0000x reference)
"""nn_ContactHead Trainium2 kernel (8-core data parallel).

out = sigmoid(w2 . relu((grid_sample(feat, uv) @ reduce_w + reduce_b) @ cls_w1 + cls_b1) + cls_b2)

Everything left of the relu is linear and bilinear sampling is linear in the
features, so the channel reductions commute with the sampling:
  W  = reduce_w @ cls_w1            (1280 x 128)   [device, PE]
  bb = reduce_b @ cls_w1 + cls_b1   (128)          [device, PE via ones-row]
  z[d, pix] = feat[:, pix].W[:, d] + bb[d]    at the 1024 pixels (PE, bf16)
Bilinear via pre-differenced pixel quantities (one gather row per vert):
  dzx = z(x+1)-z ; dzy = z(y+1)-z ; dzxy = dzy(x+1)-dzy
  v(wx,wy) = z00 + wx*dzx + wy*(dzy + wx*dzxy)
Tokens [z00|dzx|dzy|dzxy] (1KB bf16 rows, pixel-major) are written to DRAM
(PE transpose), then fetched per-vert with the batched DMAGatherAnt custom
op (one instruction per 3456 verts instead of one INDIRECT1D per 128 -
SWDGE descgen cost is ~1us fixed per op, which dominated the baseline).
The gather's index list must be int16, wrapped j%16 across partitions and
replicated across the 8 gpsimd cores; it is computed on-device from a
host-pre-wrapped replicated copy of uv.
Blend on DVE with free-dim step-0 broadcast weight APs, relu+w2 fused via
scalar_tensor_tensor, dot via tensor_reduce, sigmoid on ACT.

Vert layout: vert j lives at (partition j%128, column j//128).
"""

import ml_dtypes
import numpy as np

B, C, H, W, N = 32, 1280, 32, 32, 6890
NCORES = 8
IMGS = B // NCORES          # 4 images per core
PIX = H * W                 # 1024
PPAD = 1088                 # padded pixel slots in the dims-major z tiles
NCH = C // 128              # 10 channel chunks
MID = 128
NV = 6912                   # padded verts (= 54*128)
Q = NV // 128               # 54
NVW = 8192                  # wrapped-prep vert padding (= 16*512)
SW = NVW // 16              # 512 wrapped columns
GH = 2304                   # verts per dma_gather (third of an image)
GR = GH // 128              # 18 rows per gathered chunk tile
VROW = 9                    # rows per blend chunk (1152 verts)
TOK = 512                   # token row: 4 quantities x 128 dims (bf16)

_CACHE = {}


def _build():
    if "nc" in _CACHE:
        return _CACHE["nc"]

    from contextlib import ExitStack

    import concourse.bass as bass
    import concourse.tile as tile
    from concourse import bacc, mybir
    from concourse.ap import AP

    f32 = mybir.dt.float32
    bf16 = mybir.dt.bfloat16
    i16 = mybir.dt.int16
    OP = mybir.AluOpType
    ACT = mybir.ActivationFunctionType

    nc = bacc.Bacc("TRN2", target_bir_lowering=False, debug=False)

    feat_d = nc.dram_tensor("feat", [IMGS, C, PIX], bf16, kind="ExternalInput")
    uv_d = nc.dram_tensor("uv", [IMGS, NV, 2], f32, kind="ExternalInput")
    uvw_d = nc.dram_tensor("uvw", [IMGS, 128, 2 * SW], f32, kind="ExternalInput")
    rwt_d = nc.dram_tensor("rwt", [256, C], f32, kind="ExternalInput")
    cw1_d = nc.dram_tensor("cw1", [256, MID], f32, kind="ExternalInput")
    rb_d = nc.dram_tensor("rb", [256], f32, kind="ExternalInput")
    cb1_d = nc.dram_tensor("cb1", [MID], f32, kind="ExternalInput")
    w2r_d = nc.dram_tensor("w2r", [128, 128], f32, kind="ExternalInput")
    cb2_d = nc.dram_tensor("cb2", [128, 1], f32, kind="ExternalInput")
    id_d = nc.dram_tensor("ident", [128, 128], bf16, kind="ExternalInput")
    ztok_d = [
        nc.dram_tensor(f"ztok{i}", [PIX, TOK], bf16) for i in range(IMGS)
    ]
    out_d = nc.dram_tensor("out", [IMGS, NV], f32, kind="ExternalOutput")

    with tile.TileContext(nc) as tc, ExitStack() as ctx:
        consts = ctx.enter_context(tc.tile_pool(name="consts", bufs=1))
        featp = ctx.enter_context(tc.tile_pool(name="featp", bufs=2))
        zqp = ctx.enter_context(tc.tile_pool(name="zqp", bufs=8))
        gpool = ctx.enter_context(tc.tile_pool(name="gpool", bufs=2))
        tpool = ctx.enter_context(tc.tile_pool(name="tpool", bufs=4))
        sm = ctx.enter_context(tc.tile_pool(name="sm", bufs=4))
        wp = ctx.enter_context(tc.tile_pool(name="wp", bufs=2))
        idxp = ctx.enter_context(tc.tile_pool(name="idxp", bufs=4))
        irp = ctx.enter_context(tc.tile_pool(name="irp", bufs=4))
        lg = ctx.enter_context(tc.tile_pool(name="lg", bufs=2))

        # ---------------- phase 0: combined weights (PE) ----------------
        psw_ctx = ExitStack()
        prep = psw_ctx.enter_context(tc.tile_pool(name="prep", bufs=1))
        psw = psw_ctx.enter_context(tc.tile_pool(name="psw", bufs=2, space="PSUM"))
        rwt_t, cw1_t = [], []
        for k in range(2):
            rt = prep.tile([128, C], f32, tag=f"rwt{k}", name=f"rwt{k}")
            nc.sync.dma_start(rt[:], rwt_d.ap()[128 * k : 128 * (k + 1), :])
            rwt_t.append(rt)
            ct = prep.tile([128, MID], f32, tag=f"cw1{k}", name=f"cw1{k}")
            nc.sync.dma_start(ct[:], cw1_d.ap()[128 * k : 128 * (k + 1), :])
            cw1_t.append(ct)

        Wt = []
        for c in range(NCH):
            pw = psw.tile([128, 128], f32, tag="pw", name=f"pw{c}")
            for k in range(2):
                nc.tensor.matmul(
                    pw[:],
                    lhsT=rwt_t[k][:, 128 * c : 128 * (c + 1)],
                    rhs=cw1_t[k][:],
                    start=(k == 0),
                    stop=(k == 1),
                )
            wt = consts.tile([128, 128], bf16, tag=f"W{c}", name=f"W{c}")
            nc.scalar.copy(wt[:], pw[:])
            Wt.append(wt)

        rb_t = prep.tile([128, 2], f32, tag="rb", name="rb")
        nc.scalar.dma_start(rb_t[:], rb_d.ap().rearrange("(k p) -> p k", p=128))
        cb1_t = prep.tile([1, MID], f32, tag="cb1", name="cb1")
        nc.scalar.dma_start(cb1_t[:], cb1_d.ap().rearrange("(one d) -> one d", one=1))
        pb = psw.tile([1, 128], f32, tag="pb", name="pb")
        for k in range(2):
            nc.tensor.matmul(
                pb[:], lhsT=rb_t[:, k : k + 1], rhs=cw1_t[k][:],
                start=(k == 0), stop=(k == 1),
            )
        brow = prep.tile([1, 128], f32, tag="brow", name="brow")
        nc.vector.tensor_tensor(out=brow[:], in0=pb[:], in1=cb1_t[:], op=OP.add)
        bbias = consts.tile([1, 128], bf16, tag="bbias", name="bbias")
        nc.scalar.copy(bbias[:], brow[:])

        ones_t = consts.tile([1, PIX], bf16, tag="ones", name="ones")
        nc.vector.memset(ones_t[:], 1.0)
        ident = consts.tile([128, 128], bf16, tag="ident", name="ident")
        nc.scalar.dma_start(ident[:], id_d.ap())
        w2rf = prep.tile([128, 128], f32, tag="w2rf", name="w2rf")
        nc.scalar.dma_start(w2rf[:], w2r_d.ap())
        w2rep = consts.tile([128, 128], bf16, tag="w2rep", name="w2rep")
        nc.vector.tensor_copy(out=w2rep[:], in_=w2rf[:])
        cb2_t = consts.tile([128, 1], f32, tag="cb2", name="cb2")
        nc.scalar.dma_start(cb2_t[:], cb2_d.ap())
        psw_ctx.close()

        zps = ctx.enter_context(tc.tile_pool(name="zps", bufs=2, space="PSUM"))
        pst = ctx.enter_context(tc.tile_pool(name="pst", bufs=3, space="PSUM"))

        def emit_floor(dst, srcap, nm, cols, pool):
            """dst = floor(srcap), srcap in [0, 32); robust to convert rounding."""
            ti = pool.tile([128, cols], i16, tag=f"flt_i{cols}", name=f"fi_{nm}")
            tf = pool.tile([128, cols], f32, tag=f"flt_f{cols}", name=f"ff_{nm}")
            nc.vector.tensor_copy(out=ti[:], in_=srcap)
            nc.vector.tensor_copy(out=dst, in_=ti[:])
            nc.vector.tensor_tensor(out=tf[:], in0=dst, in1=srcap, op=OP.is_gt)
            nc.vector.tensor_tensor(out=dst, in0=dst, in1=tf[:], op=OP.subtract)

        idx16_l, wx_l, wy_l = [], [], []
        for i in range(IMGS):
            # ---------------- uv prep: blend weights (col-major layout) ----
            # vert j at (partition j%128, col j//128)
            uvt = sm.tile([128, 2 * Q], f32, tag="uvt", name=f"uvt{i}")
            uv_i = uv_d.ap()[i]
            nc.scalar.dma_start(
                uvt[:],
                AP(uv_i.tensor, uv_i.offset, [[2, 128], [256, Q], [1, 2]]),
            )
            px = sm.tile([128, Q], f32, tag="px", name=f"px{i}")
            py = sm.tile([128, Q], f32, tag="py", name=f"py{i}")
            nc.vector.tensor_scalar(out=px[:], in0=uvt[:, 0 : 2 * Q : 2],
                                    scalar1=15.5, scalar2=15.5, op0=OP.mult, op1=OP.add)
            nc.vector.tensor_scalar(out=py[:], in0=uvt[:, 1 : 2 * Q : 2],
                                    scalar1=15.5, scalar2=15.5, op0=OP.mult, op1=OP.add)
            x0 = sm.tile([128, Q], f32, tag="x0", name=f"x0{i}")
            y0 = sm.tile([128, Q], f32, tag="y0", name=f"y0{i}")
            emit_floor(x0[:], px[:], f"x{i}", Q, sm)
            emit_floor(y0[:], py[:], f"y{i}", Q, sm)
            nc.vector.tensor_scalar(out=x0[:], in0=x0[:], scalar1=30.0, scalar2=0.0,
                                    op0=OP.min, op1=OP.max)
            nc.vector.tensor_scalar(out=y0[:], in0=y0[:], scalar1=30.0, scalar2=0.0,
                                    op0=OP.min, op1=OP.max)
            wxf = sm.tile([128, Q], f32, tag="wxf", name=f"wxf{i}")
            wyf = sm.tile([128, Q], f32, tag="wyf", name=f"wyf{i}")
            nc.vector.tensor_tensor(out=wxf[:], in0=px[:], in1=x0[:], op=OP.subtract)
            nc.vector.tensor_tensor(out=wyf[:], in0=py[:], in1=y0[:], op=OP.subtract)
            wx = irp.tile([128, Q], bf16, tag="wx", name=f"wx{i}")
            wy = irp.tile([128, Q], bf16, tag="wy", name=f"wy{i}")
            nc.vector.tensor_copy(out=wx[:], in_=wxf[:])
            nc.vector.tensor_copy(out=wy[:], in_=wyf[:])
            wx_l.append(wx)
            wy_l.append(wy)

            # ---------------- wrapped idx prep for dma_gather ----------------
            # gather order g = j; idx for vert j at (partition j%16 (replicated
            # every 16), col j//16), int16.  uvw is host-pre-wrapped+replicated.
            uvwt = wp.tile([128, 2 * SW], f32, tag="uvwt", name=f"uvwt{i}")
            nc.scalar.dma_start(uvwt[:], uvw_d.ap()[i])
            pxw = wp.tile([128, SW], f32, tag="pxw", name=f"pxw{i}")
            pyw = wp.tile([128, SW], f32, tag="pyw", name=f"pyw{i}")
            nc.vector.tensor_scalar(out=pxw[:], in0=uvwt[:, 0 : 2 * SW : 2],
                                    scalar1=15.5, scalar2=15.5, op0=OP.mult, op1=OP.add)
            nc.vector.tensor_scalar(out=pyw[:], in0=uvwt[:, 1 : 2 * SW : 2],
                                    scalar1=15.5, scalar2=15.5, op0=OP.mult, op1=OP.add)
            x0w = wp.tile([128, SW], f32, tag="x0w", name=f"x0w{i}")
            y0w = wp.tile([128, SW], f32, tag="y0w", name=f"y0w{i}")
            emit_floor(x0w[:], pxw[:], f"xw{i}", SW, wp)
            emit_floor(y0w[:], pyw[:], f"yw{i}", SW, wp)
            nc.vector.tensor_scalar(out=x0w[:], in0=x0w[:], scalar1=30.0, scalar2=0.0,
                                    op0=OP.min, op1=OP.max)
            nc.vector.tensor_scalar(out=y0w[:], in0=y0w[:], scalar1=30.0, scalar2=0.0,
                                    op0=OP.min, op1=OP.max)
            idxwf = wp.tile([128, SW], f32, tag="pxw", name=f"idxwf{i}")
            nc.vector.scalar_tensor_tensor(
                out=idxwf[:], in0=y0w[:], scalar=32.0, in1=x0w[:],
                op0=OP.mult, op1=OP.add,
            )
            idx16 = idxp.tile([128, SW], i16, tag="idx16", name=f"idx16_{i}")
            nc.vector.tensor_copy(out=idx16[:], in_=idxwf[:])
            idx16_l.append(idx16)

            # ---------------- z at pixels (PE) ----------------
            # feat loaded in two 5-channel-chunk halves to halve SBUF residency
            zp = zps.tile([128, PIX], f32, tag="zp", name=f"zp{i}")
            f_i = feat_d.ap()[i]
            NH = NCH // 2
            for h in range(2):
                ft = featp.tile([128, NH * PIX], bf16, tag="ft", name=f"ft{i}_{h}")
                nc.sync.dma_start(
                    ft[:],
                    AP(f_i.tensor, f_i.offset + h * NH * 128 * PIX,
                       [[PIX, 128], [128 * PIX, NH], [1, PIX]]),
                )
                for ph in range(2):
                    sl = slice(512 * ph, 512 * (ph + 1))
                    for c5 in range(NH):
                        nc.tensor.matmul(
                            zp[:, sl],
                            lhsT=Wt[NH * h + c5][:],
                            rhs=ft[:, PIX * c5 + 512 * ph : PIX * c5 + 512 * (ph + 1)],
                            start=(h == 0 and c5 == 0),
                            stop=False,
                            skip_group_check=True,
                        )
                    if h == 1:
                        nc.tensor.matmul(
                            zp[:, sl], lhsT=bbias[:], rhs=ones_t[:, sl],
                            start=False, stop=True, skip_group_check=True,
                        )

            # escape + pre-differenced quantities (dims-major, bf16)
            zq = zqp.tile([128, PPAD], bf16, tag="zq", name=f"zq{i}")
            dzx = zqp.tile([128, PPAD], bf16, tag="zq", name=f"dzx{i}")
            dzy = zqp.tile([128, PPAD], bf16, tag="zq", name=f"dzy{i}")
            dzxy = zqp.tile([128, PPAD], bf16, tag="zq", name=f"dzxy{i}")
            nc.scalar.copy(zq[:, 0:PIX], zp[:])
            nc.vector.memset(zq[:, PIX:PPAD], 0.0)
            nc.vector.tensor_tensor(out=dzx[:, 0:1056], in0=zq[:, 1:1057],
                                    in1=zq[:, 0:1056], op=OP.subtract)
            nc.vector.memset(dzx[:, 1056:PPAD], 0.0)
            nc.vector.tensor_tensor(out=dzy[:, 0:1056], in0=zq[:, 32:PPAD],
                                    in1=zq[:, 0:1056], op=OP.subtract)
            nc.vector.memset(dzy[:, 1056:PPAD], 0.0)
            nc.vector.tensor_tensor(out=dzxy[:, 0:1055], in0=dzy[:, 1:1056],
                                    in1=dzy[:, 0:1055], op=OP.subtract)
            nc.vector.memset(dzxy[:, 1055:PPAD], 0.0)

            # ---------------- tokens to DRAM (PE transpose per 128-pix block) ----
            stg = featp.tile([128, 8 * TOK], bf16, tag="stg", name=f"stg{i}")
            for b in range(8):
                pt = pst.tile([128, TOK], bf16, tag="pt", name=f"pt{i}_{b}")
                for qi, zt in enumerate((zq, dzx, dzy, dzxy)):
                    nc.tensor.transpose(
                        pt[:, 128 * qi : 128 * (qi + 1)],
                        zt[:, 128 * b : 128 * (b + 1)],
                        ident[:],
                    )
                nc.scalar.copy(stg[:, TOK * b : TOK * (b + 1)], pt[:])
            zt_i = ztok_d[i].ap()
            nc.sync.dma_start(
                AP(zt_i.tensor, zt_i.offset,
                   [[TOK, 128], [128 * TOK, 8], [1, TOK]]),
                stg[:].rearrange("p (b t) -> p b t", t=TOK),
            )


        for i in range(IMGS):
            # ---------------- gather + blend + dot per 2304-vert chunk ----------
            logit = lg.tile([128, Q], f32, tag="logit", name=f"lg{i}")
            NIC = GH // 16          # idx cols per gather
            for gk in range(3):
                gt = gpool.tile([128, GR * TOK], bf16, tag="g", name=f"g{i}_{gk}")
                g3full = gt[:].rearrange("p (r t) -> p r t", t=TOK)
                nc.gpsimd.dma_gather(
                    out_ap=g3full,
                    in_ap=ztok_d[i].ap(),
                    idxs_ap=idx16_l[i][:, NIC * gk : NIC * (gk + 1)],
                    num_idxs=GH,
                    num_idxs_reg=GH,
                    elem_size=TOK,
                    single_packet=False,
                )

                for sub in range(2):
                    ck = 2 * gk + sub
                    g3 = gt[:].rearrange("p (r t) -> p r t", t=TOK)[
                        :, VROW * sub : VROW * (sub + 1), :
                    ]

                    def wap(wtile, ck=ck):
                        a = wtile[:]
                        return AP(
                            a.tensor,
                            a.offset + VROW * ck * a.ap[-1][0],
                            [[a.ap[0][0], 128], [a.ap[-1][0], VROW], [0, 128]],
                        )

                    t1 = tpool.tile([128, VROW * 128], bf16, tag="t1", name=f"t1_{i}_{ck}")
                    t13 = t1[:].rearrange("p (r d) -> p r d", d=128)
                    acc = tpool.tile([128, VROW * 128], bf16, tag="acc", name=f"ac{i}_{ck}")
                    acc3 = acc[:].rearrange("p (r d) -> p r d", d=128)
                    # t1 = wx*dzx ; acc = z00 + t1
                    nc.vector.tensor_tensor(out=t13, in0=g3[:, :, 128:256], in1=wap(wx_l[i]), op=OP.mult)
                    nc.vector.tensor_tensor(out=acc3, in0=g3[:, :, 0:128], in1=t13, op=OP.add)
                    # t1 = wx*dzxy ; t1 += dzy ; t1 *= wy ; acc += t1
                    nc.vector.tensor_tensor(out=t13, in0=g3[:, :, 384:512], in1=wap(wx_l[i]), op=OP.mult)
                    nc.vector.tensor_tensor(out=t13, in0=g3[:, :, 256:384], in1=t13, op=OP.add)
                    nc.vector.tensor_tensor(out=t13, in0=t13, in1=wap(wy_l[i]), op=OP.mult)
                    nc.vector.tensor_tensor(out=acc3, in0=acc3, in1=t13, op=OP.add)
                    # h = relu(acc) * w2   (fused), then reduce over dims
                    w2ap = AP(
                        w2rep[:].tensor, w2rep[:].offset,
                        [[w2rep[:].ap[0][0], 128], [0, VROW], [1, 128]],
                    )
                    nc.vector.scalar_tensor_tensor(
                        out=acc3, in0=acc3, scalar=0.0, in1=w2ap,
                        op0=OP.max, op1=OP.mult,
                    )
                    nc.vector.tensor_reduce(
                        out=logit[:, VROW * ck : VROW * (ck + 1)].rearrange(
                            "p (r one) -> p r one", one=1
                        ),
                        in_=acc3,
                        axis=mybir.AxisListType.X,
                        op=OP.add,
                    )
            ostg = lg.tile([128, Q], f32, tag="ostg", name=f"os{i}")
            nc.scalar.activation(ostg[:], logit[:], ACT.Sigmoid, bias=cb2_t[:])
            o_i = out_d.ap()[i]
            oap = AP(o_i.tensor, o_i.offset, [[1, 128], [128, Q]])
            nc.scalar.dma_start(oap, ostg[:])

    nc.compile()
    _CACHE["nc"] = nc
    return nc


def _host_prep(inputs):
    feat = np.asarray(inputs["feat_map"], dtype=np.float32)
    uv = np.asarray(inputs["verts_uv"], dtype=np.float32)
    rw = np.asarray(inputs["reduce_w"], dtype=np.float32)
    rb = np.asarray(inputs["reduce_b"], dtype=np.float32)
    w1 = np.asarray(inputs["cls_w1"], dtype=np.float32)
    b1 = np.asarray(inputs["cls_b1"], dtype=np.float32)
    w2 = np.asarray(inputs["cls_w2"], dtype=np.float32)
    b2 = np.asarray(inputs["cls_b2"], dtype=np.float32)

    rwt = np.ascontiguousarray(rw.T)                      # (256, 1280)
    uvp = np.zeros((B, NV, 2), dtype=np.float32)
    uvp[:, :N, :] = uv
    # wrapped copy for the gather-index prep: vert j at (partition j%16
    # replicated every 16, col j//16); padded to 8192 verts.
    uvw = np.zeros((B, NVW, 2), dtype=np.float32)
    uvw[:, :N, :] = uv
    uvw = uvw.reshape(B, SW, 16, 2).transpose(0, 2, 1, 3)   # (B,16,512,2)
    uvw = np.tile(uvw, (1, 8, 1, 1)).reshape(B, 128, 2 * SW)
    featr = feat.reshape(B, C, PIX).astype(ml_dtypes.bfloat16)

    shared = {
        "rwt": rwt,
        "cw1": np.ascontiguousarray(w1),
        "rb": rb,
        "cb1": b1,
        "w2r": np.ascontiguousarray(np.tile(w2[None, :], (128, 1))),
        "cb2": np.full((128, 1), b2[0], dtype=np.float32),
        "ident": np.eye(128, dtype=ml_dtypes.bfloat16),
    }
    in_maps = []
    for core in range(NCORES):
        sl = slice(core * IMGS, (core + 1) * IMGS)
        m = dict(shared)
        m["feat"] = np.ascontiguousarray(featr[sl])
        m["uv"] = np.ascontiguousarray(uvp[sl])
        m["uvw"] = np.ascontiguousarray(uvw[sl])
        in_maps.append(m)
    return in_maps


def kernel(**inputs):
    from concourse.bass_utils import run_bass_kernel_spmd

    nc = _build()
    in_maps = _host_prep(inputs)
    res = run_bass_kernel_spmd(nc, in_maps, list(range(NCORES)))
    out = np.empty((B, N), dtype=np.float32)
    for core in range(NCORES):
        dev = res.results[core]["out"]          # (IMGS, NV), vert j at col j
        out[core * IMGS : (core + 1) * IMGS] = dev[:, :N]
    return out


# revision 15
# speedup vs baseline: 1.1125x; 1.1125x over previous
"""nn_ContactHead Trainium2 kernel (8-core data parallel).

out = sigmoid(w2 . relu((grid_sample(feat, uv) @ reduce_w + reduce_b) @ cls_w1 + cls_b1) + cls_b2)

Everything left of the relu is linear and bilinear sampling is linear in the
features, so the channel reductions commute with the sampling:
  W  = reduce_w @ cls_w1            (1280 x 128)   [device, PE]
  bb = reduce_b @ cls_w1 + cls_b1   (128)          [device, PE via ones-row]
  z[d, pix] = feat[:, pix].W[:, d] + bb[d]    at the 1024 pixels (PE, bf16)
Bilinear via pre-differenced pixel quantities (one gather row per vert):
  dzx = z(x+1)-z ; dzy = z(y+1)-z ; dzxy = dzy(x+1)-dzy
  v(wx,wy) = z00 + wx*dzx + wy*(dzy + wx*dzxy)
Tokens [z00|dzx|dzy|dzxy] (1KB bf16 rows, pixel-major) are written to DRAM
(PE transpose), then fetched per-vert with the batched DMAGatherAnt custom
op (one instruction per 3456 verts instead of one INDIRECT1D per 128 -
SWDGE descgen cost is ~1us fixed per op, which dominated the baseline).
The gather's index list must be int16, wrapped j%16 across partitions and
replicated across the 8 gpsimd cores; it is computed on-device from a
host-pre-wrapped replicated copy of uv.
Blend on DVE with free-dim step-0 broadcast weight APs, relu+w2 fused via
scalar_tensor_tensor, dot via tensor_reduce, sigmoid on ACT.

Vert layout: vert j lives at (partition j%128, column j//128).
"""

import ml_dtypes
import numpy as np

B, C, H, W, N = 32, 1280, 32, 32, 6890
NCORES = 8
IMGS = B // NCORES          # 4 images per core
PIX = H * W                 # 1024
PPAD = 1088                 # padded pixel slots in the dims-major z tiles
NCH = C // 128              # 10 channel chunks
MID = 128
NV = 6912                   # padded verts (= 54*128)
Q = NV // 128               # 54
NVW = 8192                  # wrapped-prep vert padding (= 16*512)
SW = NVW // 16              # 512 wrapped columns
GH = 2304                   # verts per dma_gather (third of an image)
GR = GH // 128              # 18 rows per gathered chunk tile
VROW = 9                    # rows per blend chunk (1152 verts)
TOK = 512                   # token row: 4 quantities x 128 dims (bf16)

_CACHE = {}


def _build():
    if "nc" in _CACHE:
        return _CACHE["nc"]

    from contextlib import ExitStack

    import concourse.bass as bass
    import concourse.tile as tile
    from concourse import bacc, mybir
    from concourse.ap import AP

    f32 = mybir.dt.float32
    bf16 = mybir.dt.bfloat16
    i16 = mybir.dt.int16
    OP = mybir.AluOpType
    ACT = mybir.ActivationFunctionType

    nc = bacc.Bacc("TRN2", target_bir_lowering=False, debug=False)

    feat_d = nc.dram_tensor("feat", [IMGS, C, PIX], bf16, kind="ExternalInput")
    uv_d = nc.dram_tensor("uv", [IMGS, NV, 2], f32, kind="ExternalInput")
    uvw_d = nc.dram_tensor("uvw", [IMGS, 128, 2 * SW], f32, kind="ExternalInput")
    rwt_d = nc.dram_tensor("rwt", [256, C], f32, kind="ExternalInput")
    cw1_d = nc.dram_tensor("cw1", [256, MID], f32, kind="ExternalInput")
    rb_d = nc.dram_tensor("rb", [256], f32, kind="ExternalInput")
    cb1_d = nc.dram_tensor("cb1", [MID], f32, kind="ExternalInput")
    w2r_d = nc.dram_tensor("w2r", [128, 128], f32, kind="ExternalInput")
    cb2_d = nc.dram_tensor("cb2", [128, 1], f32, kind="ExternalInput")
    id_d = nc.dram_tensor("ident", [128, 128], bf16, kind="ExternalInput")
    ztok_d = [
        nc.dram_tensor(f"ztok{i}", [PIX, TOK], bf16) for i in range(IMGS)
    ]
    out_d = nc.dram_tensor("out", [IMGS, NV], f32, kind="ExternalOutput")

    with tile.TileContext(nc) as tc, ExitStack() as ctx:
        consts = ctx.enter_context(tc.tile_pool(name="consts", bufs=1))
        featp = ctx.enter_context(tc.tile_pool(name="featp", bufs=2))
        zqp = ctx.enter_context(tc.tile_pool(name="zqp", bufs=8))
        gpool = ctx.enter_context(tc.tile_pool(name="gpool", bufs=3))
        tpool = ctx.enter_context(tc.tile_pool(name="tpool", bufs=4))
        sm = ctx.enter_context(tc.tile_pool(name="sm", bufs=4))
        wp = ctx.enter_context(tc.tile_pool(name="wp", bufs=2))
        idxp = ctx.enter_context(tc.tile_pool(name="idxp", bufs=4))
        irp = ctx.enter_context(tc.tile_pool(name="irp", bufs=4))
        lg = ctx.enter_context(tc.tile_pool(name="lg", bufs=2))

        # ---------------- phase 0: combined weights (PE) ----------------
        psw_ctx = ExitStack()
        prep = psw_ctx.enter_context(tc.tile_pool(name="prep", bufs=1))
        psw = psw_ctx.enter_context(tc.tile_pool(name="psw", bufs=2, space="PSUM"))
        rwt_t, cw1_t = [], []
        for k in range(2):
            rt = prep.tile([128, C], f32, tag=f"rwt{k}", name=f"rwt{k}")
            nc.sync.dma_start(rt[:], rwt_d.ap()[128 * k : 128 * (k + 1), :])
            rwt_t.append(rt)
            ct = prep.tile([128, MID], f32, tag=f"cw1{k}", name=f"cw1{k}")
            nc.sync.dma_start(ct[:], cw1_d.ap()[128 * k : 128 * (k + 1), :])
            cw1_t.append(ct)

        Wt = []
        for c in range(NCH):
            pw = psw.tile([128, 128], f32, tag="pw", name=f"pw{c}")
            for k in range(2):
                nc.tensor.matmul(
                    pw[:],
                    lhsT=rwt_t[k][:, 128 * c : 128 * (c + 1)],
                    rhs=cw1_t[k][:],
                    start=(k == 0),
                    stop=(k == 1),
                )
            wt = consts.tile([128, 128], bf16, tag=f"W{c}", name=f"W{c}")
            nc.scalar.copy(wt[:], pw[:])
            Wt.append(wt)

        rb_t = prep.tile([128, 2], f32, tag="rb", name="rb")
        nc.scalar.dma_start(rb_t[:], rb_d.ap().rearrange("(k p) -> p k", p=128))
        cb1_t = prep.tile([1, MID], f32, tag="cb1", name="cb1")
        nc.scalar.dma_start(cb1_t[:], cb1_d.ap().rearrange("(one d) -> one d", one=1))
        pb = psw.tile([1, 128], f32, tag="pb", name="pb")
        for k in range(2):
            nc.tensor.matmul(
                pb[:], lhsT=rb_t[:, k : k + 1], rhs=cw1_t[k][:],
                start=(k == 0), stop=(k == 1),
            )
        brow = prep.tile([1, 128], f32, tag="brow", name="brow")
        nc.vector.tensor_tensor(out=brow[:], in0=pb[:], in1=cb1_t[:], op=OP.add)
        bbias = consts.tile([1, 128], bf16, tag="bbias", name="bbias")
        nc.scalar.copy(bbias[:], brow[:])

        ones_t = consts.tile([1, PIX], bf16, tag="ones", name="ones")
        nc.vector.memset(ones_t[:], 1.0)
        ident = consts.tile([128, 128], bf16, tag="ident", name="ident")
        nc.scalar.dma_start(ident[:], id_d.ap())
        w2rf = prep.tile([128, 128], f32, tag="w2rf", name="w2rf")
        nc.scalar.dma_start(w2rf[:], w2r_d.ap())
        w2rep = consts.tile([128, 128], bf16, tag="w2rep", name="w2rep")
        nc.vector.tensor_copy(out=w2rep[:], in_=w2rf[:])
        cb2_t = consts.tile([128, 1], f32, tag="cb2", name="cb2")
        nc.scalar.dma_start(cb2_t[:], cb2_d.ap())
        psw_ctx.close()

        zps = ctx.enter_context(tc.tile_pool(name="zps", bufs=2, space="PSUM"))
        pst = ctx.enter_context(tc.tile_pool(name="pst", bufs=3, space="PSUM"))

        def emit_floor(dst, srcap, nm, cols, pool):
            """dst = floor(srcap), srcap in [0, 32); robust to convert rounding."""
            ti = pool.tile([128, cols], i16, tag=f"flt_i{cols}", name=f"fi_{nm}")
            tf = pool.tile([128, cols], f32, tag=f"flt_f{cols}", name=f"ff_{nm}")
            nc.vector.tensor_copy(out=ti[:], in_=srcap)
            nc.vector.tensor_copy(out=dst, in_=ti[:])
            nc.vector.tensor_tensor(out=tf[:], in0=dst, in1=srcap, op=OP.is_gt)
            nc.vector.tensor_tensor(out=dst, in0=dst, in1=tf[:], op=OP.subtract)

        idx16_l, wx_l, wy_l = [], [], []
        for i in range(IMGS):
            # ---------------- wrapped idx prep for dma_gather ----------------
            # gather order g = j; idx for vert j at (partition j%16 (replicated
            # every 16), col j//16), int16.  uvw is host-pre-wrapped+replicated.
            uvwt = wp.tile([128, 2 * SW], f32, tag="uvwt", name=f"uvwt{i}")
            nc.scalar.dma_start(uvwt[:], uvw_d.ap()[i])
            pxw = wp.tile([128, SW], f32, tag="pxw", name=f"pxw{i}")
            pyw = wp.tile([128, SW], f32, tag="pyw", name=f"pyw{i}")
            nc.vector.tensor_scalar(out=pxw[:], in0=uvwt[:, 0 : 2 * SW : 2],
                                    scalar1=15.5, scalar2=15.5, op0=OP.mult, op1=OP.add)
            nc.vector.tensor_scalar(out=pyw[:], in0=uvwt[:, 1 : 2 * SW : 2],
                                    scalar1=15.5, scalar2=15.5, op0=OP.mult, op1=OP.add)
            x0w = wp.tile([128, SW], f32, tag="x0w", name=f"x0w{i}")
            y0w = wp.tile([128, SW], f32, tag="y0w", name=f"y0w{i}")
            emit_floor(x0w[:], pxw[:], f"xw{i}", SW, wp)
            emit_floor(y0w[:], pyw[:], f"yw{i}", SW, wp)
            nc.vector.tensor_scalar(out=x0w[:], in0=x0w[:], scalar1=30.0, scalar2=0.0,
                                    op0=OP.min, op1=OP.max)
            nc.vector.tensor_scalar(out=y0w[:], in0=y0w[:], scalar1=30.0, scalar2=0.0,
                                    op0=OP.min, op1=OP.max)
            idxwf = wp.tile([128, SW], f32, tag="pxw", name=f"idxwf{i}")
            nc.vector.scalar_tensor_tensor(
                out=idxwf[:], in0=y0w[:], scalar=32.0, in1=x0w[:],
                op0=OP.mult, op1=OP.add,
            )
            idx16 = idxp.tile([128, SW], i16, tag="idx16", name=f"idx16_{i}")
            nc.vector.tensor_copy(out=idx16[:], in_=idxwf[:])
            idx16_l.append(idx16)

            # ---------------- uv prep: blend weights (col-major layout) ----
            # vert j at (partition j%128, col j//128)
            uvt = sm.tile([128, 2 * Q], f32, tag="uvt", name=f"uvt{i}")
            uv_i = uv_d.ap()[i]
            nc.scalar.dma_start(
                uvt[:],
                AP(uv_i.tensor, uv_i.offset, [[2, 128], [256, Q], [1, 2]]),
            )
            px = sm.tile([128, Q], f32, tag="px", name=f"px{i}")
            py = sm.tile([128, Q], f32, tag="py", name=f"py{i}")
            nc.vector.tensor_scalar(out=px[:], in0=uvt[:, 0 : 2 * Q : 2],
                                    scalar1=15.5, scalar2=15.5, op0=OP.mult, op1=OP.add)
            nc.vector.tensor_scalar(out=py[:], in0=uvt[:, 1 : 2 * Q : 2],
                                    scalar1=15.5, scalar2=15.5, op0=OP.mult, op1=OP.add)
            x0 = sm.tile([128, Q], f32, tag="x0", name=f"x0{i}")
            y0 = sm.tile([128, Q], f32, tag="y0", name=f"y0{i}")
            emit_floor(x0[:], px[:], f"x{i}", Q, sm)
            emit_floor(y0[:], py[:], f"y{i}", Q, sm)
            nc.vector.tensor_scalar(out=x0[:], in0=x0[:], scalar1=30.0, scalar2=0.0,
                                    op0=OP.min, op1=OP.max)
            nc.vector.tensor_scalar(out=y0[:], in0=y0[:], scalar1=30.0, scalar2=0.0,
                                    op0=OP.min, op1=OP.max)
            wxf = sm.tile([128, Q], f32, tag="wxf", name=f"wxf{i}")
            wyf = sm.tile([128, Q], f32, tag="wyf", name=f"wyf{i}")
            nc.vector.tensor_tensor(out=wxf[:], in0=px[:], in1=x0[:], op=OP.subtract)
            nc.vector.tensor_tensor(out=wyf[:], in0=py[:], in1=y0[:], op=OP.subtract)
            wx = irp.tile([128, Q], bf16, tag="wx", name=f"wx{i}")
            wy = irp.tile([128, Q], bf16, tag="wy", name=f"wy{i}")
            nc.vector.tensor_copy(out=wx[:], in_=wxf[:])
            nc.vector.tensor_copy(out=wy[:], in_=wyf[:])
            wx_l.append(wx)
            wy_l.append(wy)

            # ---------------- z at pixels (PE) ----------------
            # feat loaded in two 5-channel-chunk halves to halve SBUF residency
            zp = zps.tile([128, PIX], f32, tag="zp", name=f"zp{i}")
            f_i = feat_d.ap()[i]
            NH = NCH // 2
            for h in range(2):
                ft = featp.tile([128, NH * PIX], bf16, tag="ft", name=f"ft{i}_{h}")
                nc.sync.dma_start(
                    ft[:],
                    AP(f_i.tensor, f_i.offset + h * NH * 128 * PIX,
                       [[PIX, 128], [128 * PIX, NH], [1, PIX]]),
                )
                for ph in range(2):
                    sl = slice(512 * ph, 512 * (ph + 1))
                    for c5 in range(NH):
                        nc.tensor.matmul(
                            zp[:, sl],
                            lhsT=Wt[NH * h + c5][:],
                            rhs=ft[:, PIX * c5 + 512 * ph : PIX * c5 + 512 * (ph + 1)],
                            start=(h == 0 and c5 == 0),
                            stop=False,
                            skip_group_check=True,
                        )
                    if h == 1:
                        nc.tensor.matmul(
                            zp[:, sl], lhsT=bbias[:], rhs=ones_t[:, sl],
                            start=False, stop=True, skip_group_check=True,
                        )

            # escape + pre-differenced quantities (dims-major, bf16)
            zq = zqp.tile([128, PPAD], bf16, tag="zq", name=f"zq{i}")
            dzx = zqp.tile([128, PPAD], bf16, tag="zq", name=f"dzx{i}")
            dzy = zqp.tile([128, PPAD], bf16, tag="zq", name=f"dzy{i}")
            dzxy = zqp.tile([128, PPAD], bf16, tag="zq", name=f"dzxy{i}")
            nc.scalar.copy(zq[:, 0:PIX], zp[:])
            nc.vector.memset(zq[:, PIX:PPAD], 0.0)
            nc.vector.tensor_tensor(out=dzx[:, 0:1056], in0=zq[:, 1:1057],
                                    in1=zq[:, 0:1056], op=OP.subtract)
            nc.vector.memset(dzx[:, 1056:PPAD], 0.0)
            nc.vector.tensor_tensor(out=dzy[:, 0:1056], in0=zq[:, 32:PPAD],
                                    in1=zq[:, 0:1056], op=OP.subtract)
            nc.vector.memset(dzy[:, 1056:PPAD], 0.0)
            nc.vector.tensor_tensor(out=dzxy[:, 0:1055], in0=dzy[:, 1:1056],
                                    in1=dzy[:, 0:1055], op=OP.subtract)
            nc.vector.memset(dzxy[:, 1055:PPAD], 0.0)

            # ---------------- tokens to DRAM (PE transpose per 128-pix block) ----
            stg = featp.tile([128, 8 * TOK], bf16, tag="stg", name=f"stg{i}")
            for b in range(8):
                pt = pst.tile([128, TOK], bf16, tag="pt", name=f"pt{i}_{b}")
                for qi, zt in enumerate((zq, dzx, dzy, dzxy)):
                    nc.tensor.transpose(
                        pt[:, 128 * qi : 128 * (qi + 1)],
                        zt[:, 128 * b : 128 * (b + 1)],
                        ident[:],
                    )
                nc.scalar.copy(stg[:, TOK * b : TOK * (b + 1)], pt[:])
            zt_i = ztok_d[i].ap()
            nc.sync.dma_start(
                AP(zt_i.tensor, zt_i.offset,
                   [[TOK, 128], [128 * TOK, 8], [1, TOK]]),
                stg[:].rearrange("p (b t) -> p b t", t=TOK),
            )


        for i in range(IMGS):
            # ---------------- gather + blend + dot per 2304-vert chunk ----------
            logit = lg.tile([128, Q], f32, tag="logit", name=f"lg{i}")
            NIC = GH // 16          # idx cols per gather
            for gk in range(3):
                gt = gpool.tile([128, GR * TOK], bf16, tag="g", name=f"g{i}_{gk}")
                g3full = gt[:].rearrange("p (r t) -> p r t", t=TOK)
                nc.gpsimd.dma_gather(
                    out_ap=g3full,
                    in_ap=ztok_d[i].ap(),
                    idxs_ap=idx16_l[i][:, NIC * gk : NIC * (gk + 1)],
                    num_idxs=GH,
                    num_idxs_reg=GH,
                    elem_size=TOK,
                    single_packet=False,
                )

                for sub in range(2):
                    ck = 2 * gk + sub
                    g3 = gt[:].rearrange("p (r t) -> p r t", t=TOK)[
                        :, VROW * sub : VROW * (sub + 1), :
                    ]

                    def wap(wtile, ck=ck):
                        a = wtile[:]
                        return AP(
                            a.tensor,
                            a.offset + VROW * ck * a.ap[-1][0],
                            [[a.ap[0][0], 128], [a.ap[-1][0], VROW], [0, 128]],
                        )

                    t1 = tpool.tile([128, VROW * 128], bf16, tag="t1", name=f"t1_{i}_{ck}")
                    t13 = t1[:].rearrange("p (r d) -> p r d", d=128)
                    acc = tpool.tile([128, VROW * 128], bf16, tag="acc", name=f"ac{i}_{ck}")
                    acc3 = acc[:].rearrange("p (r d) -> p r d", d=128)
                    # t1 = wx*dzx ; acc = z00 + t1
                    nc.vector.tensor_tensor(out=t13, in0=g3[:, :, 128:256], in1=wap(wx_l[i]), op=OP.mult)
                    nc.vector.tensor_tensor(out=acc3, in0=g3[:, :, 0:128], in1=t13, op=OP.add)
                    # t1 = wx*dzxy ; t1 += dzy ; t1 *= wy ; acc += t1
                    nc.vector.tensor_tensor(out=t13, in0=g3[:, :, 384:512], in1=wap(wx_l[i]), op=OP.mult)
                    nc.vector.tensor_tensor(out=t13, in0=g3[:, :, 256:384], in1=t13, op=OP.add)
                    nc.vector.tensor_tensor(out=t13, in0=t13, in1=wap(wy_l[i]), op=OP.mult)
                    nc.vector.tensor_tensor(out=acc3, in0=acc3, in1=t13, op=OP.add)
                    # h = relu(acc) * w2   (fused), then reduce over dims
                    w2ap = AP(
                        w2rep[:].tensor, w2rep[:].offset,
                        [[w2rep[:].ap[0][0], 128], [0, VROW], [1, 128]],
                    )
                    nc.vector.scalar_tensor_tensor(
                        out=acc3, in0=acc3, scalar=0.0, in1=w2ap,
                        op0=OP.max, op1=OP.mult,
                    )
                    nc.vector.tensor_reduce(
                        out=logit[:, VROW * ck : VROW * (ck + 1)].rearrange(
                            "p (r one) -> p r one", one=1
                        ),
                        in_=acc3,
                        axis=mybir.AxisListType.X,
                        op=OP.add,
                    )
            ostg = lg.tile([128, Q], f32, tag="ostg", name=f"os{i}")
            nc.scalar.activation(ostg[:], logit[:], ACT.Sigmoid, bias=cb2_t[:])
            o_i = out_d.ap()[i]
            oap = AP(o_i.tensor, o_i.offset, [[1, 128], [128, Q]])
            nc.scalar.dma_start(oap, ostg[:])

    nc.compile()
    _CACHE["nc"] = nc
    return nc


def _host_prep(inputs):
    feat = np.asarray(inputs["feat_map"], dtype=np.float32)
    uv = np.asarray(inputs["verts_uv"], dtype=np.float32)
    rw = np.asarray(inputs["reduce_w"], dtype=np.float32)
    rb = np.asarray(inputs["reduce_b"], dtype=np.float32)
    w1 = np.asarray(inputs["cls_w1"], dtype=np.float32)
    b1 = np.asarray(inputs["cls_b1"], dtype=np.float32)
    w2 = np.asarray(inputs["cls_w2"], dtype=np.float32)
    b2 = np.asarray(inputs["cls_b2"], dtype=np.float32)

    rwt = np.ascontiguousarray(rw.T)                      # (256, 1280)
    uvp = np.zeros((B, NV, 2), dtype=np.float32)
    uvp[:, :N, :] = uv
    # wrapped copy for the gather-index prep: vert j at (partition j%16
    # replicated every 16, col j//16); padded to 8192 verts.
    uvw = np.zeros((B, NVW, 2), dtype=np.float32)
    uvw[:, :N, :] = uv
    uvw = uvw.reshape(B, SW, 16, 2).transpose(0, 2, 1, 3)   # (B,16,512,2)
    uvw = np.tile(uvw, (1, 8, 1, 1)).reshape(B, 128, 2 * SW)
    featr = feat.reshape(B, C, PIX).astype(ml_dtypes.bfloat16)

    shared = {
        "rwt": rwt,
        "cw1": np.ascontiguousarray(w1),
        "rb": rb,
        "cb1": b1,
        "w2r": np.ascontiguousarray(np.tile(w2[None, :], (128, 1))),
        "cb2": np.full((128, 1), b2[0], dtype=np.float32),
        "ident": np.eye(128, dtype=ml_dtypes.bfloat16),
    }
    in_maps = []
    for core in range(NCORES):
        sl = slice(core * IMGS, (core + 1) * IMGS)
        m = dict(shared)
        m["feat"] = np.ascontiguousarray(featr[sl])
        m["uv"] = np.ascontiguousarray(uvp[sl])
        m["uvw"] = np.ascontiguousarray(uvw[sl])
        in_maps.append(m)
    return in_maps


def kernel(**inputs):
    from concourse.bass_utils import run_bass_kernel_spmd

    nc = _build()
    in_maps = _host_prep(inputs)
    res = run_bass_kernel_spmd(nc, in_maps, list(range(NCORES)))
    out = np.empty((B, N), dtype=np.float32)
    for core in range(NCORES):
        dev = res.results[core]["out"]          # (IMGS, NV), vert j at col j
        out[core * IMGS : (core + 1) * IMGS] = dev[:, :N]
    return out


# revision 16
# speedup vs baseline: 1.1652x; 1.0473x over previous
"""nn_ContactHead Trainium2 kernel (8-core data parallel).

out = sigmoid(w2 . relu((grid_sample(feat, uv) @ reduce_w + reduce_b) @ cls_w1 + cls_b1) + cls_b2)

Everything left of the relu is linear and bilinear sampling is linear in the
features, so the channel reductions commute with the sampling:
  W  = reduce_w @ cls_w1            (1280 x 128)   [device, PE]
  bb = reduce_b @ cls_w1 + cls_b1   (128)          [device, PE via ones-row]
  z[d, pix] = feat[:, pix].W[:, d] + bb[d]    at the 1024 pixels (PE, bf16)
Bilinear via pre-differenced pixel quantities (one gather row per vert):
  dzx = z(x+1)-z ; dzy = z(y+1)-z ; dzxy = dzy(x+1)-dzy
  v(wx,wy) = z00 + wx*dzx + wy*(dzy + wx*dzxy)
Tokens [z00|dzx|dzy|dzxy] (1KB bf16 rows, pixel-major) are written to DRAM
(PE transpose), then fetched per-vert with the batched DMAGatherAnt custom
op (one instruction per 3456 verts instead of one INDIRECT1D per 128 -
SWDGE descgen cost is ~1us fixed per op, which dominated the baseline).
The gather's index list must be int16, wrapped j%16 across partitions and
replicated across the 8 gpsimd cores; it is computed on-device from a
host-pre-wrapped replicated copy of uv.
Blend on DVE with free-dim step-0 broadcast weight APs, relu+w2 fused via
scalar_tensor_tensor, dot via tensor_reduce, sigmoid on ACT.

Vert layout: vert j lives at (partition j%128, column j//128).
"""

import ml_dtypes
import numpy as np

B, C, H, W, N = 32, 1280, 32, 32, 6890
NCORES = 8
IMGS = B // NCORES          # 4 images per core
PIX = H * W                 # 1024
PPAD = 1088                 # padded pixel slots in the dims-major z tiles
NCH = C // 128              # 10 channel chunks
MID = 128
NV = 6912                   # padded verts (= 54*128)
Q = NV // 128               # 54
NVW = 8192                  # wrapped-prep vert padding (= 16*512)
SW = NVW // 16              # 512 wrapped columns
GH = 2304                   # verts per dma_gather (third of an image)
GR = GH // 128              # 18 rows per gathered chunk tile
VROW = 9                    # rows per blend chunk (1152 verts)
TOK = 512                   # token row: 4 quantities x 128 dims (bf16)

_CACHE = {}


def _build():
    if "nc" in _CACHE:
        return _CACHE["nc"]

    from contextlib import ExitStack

    import concourse.bass as bass
    import concourse.tile as tile
    from concourse import bacc, mybir
    from concourse.ap import AP

    f32 = mybir.dt.float32
    bf16 = mybir.dt.bfloat16
    i16 = mybir.dt.int16
    OP = mybir.AluOpType
    ACT = mybir.ActivationFunctionType

    nc = bacc.Bacc("TRN2", target_bir_lowering=False, debug=False)

    feat_d = nc.dram_tensor("feat", [IMGS, C, PIX], bf16, kind="ExternalInput")
    uv_d = nc.dram_tensor("uv", [IMGS, NV, 2], f32, kind="ExternalInput")
    uvw_d = nc.dram_tensor("uvw", [IMGS, 128, 2 * SW], f32, kind="ExternalInput")
    rwt_d = nc.dram_tensor("rwt", [256, C], f32, kind="ExternalInput")
    cw1_d = nc.dram_tensor("cw1", [256, MID], f32, kind="ExternalInput")
    rb_d = nc.dram_tensor("rb", [256], f32, kind="ExternalInput")
    cb1_d = nc.dram_tensor("cb1", [MID], f32, kind="ExternalInput")
    w2r_d = nc.dram_tensor("w2r", [128, 128], f32, kind="ExternalInput")
    cb2_d = nc.dram_tensor("cb2", [128, 1], f32, kind="ExternalInput")
    id_d = nc.dram_tensor("ident", [128, 128], bf16, kind="ExternalInput")
    ztok_d = [
        nc.dram_tensor(f"ztok{i}", [PIX, TOK], bf16) for i in range(IMGS)
    ]
    out_d = nc.dram_tensor("out", [IMGS, NV], f32, kind="ExternalOutput")

    with tile.TileContext(nc) as tc, ExitStack() as ctx:
        consts = ctx.enter_context(tc.tile_pool(name="consts", bufs=1))
        featp = ctx.enter_context(tc.tile_pool(name="featp", bufs=2))
        zqp = ctx.enter_context(tc.tile_pool(name="zqp", bufs=8))
        gpool = ctx.enter_context(tc.tile_pool(name="gpool", bufs=3))
        tpool = ctx.enter_context(tc.tile_pool(name="tpool", bufs=4))
        sm = ctx.enter_context(tc.tile_pool(name="sm", bufs=4))
        wp = ctx.enter_context(tc.tile_pool(name="wp", bufs=2))
        idxp = ctx.enter_context(tc.tile_pool(name="idxp", bufs=4))
        irp = ctx.enter_context(tc.tile_pool(name="irp", bufs=4))
        lg = ctx.enter_context(tc.tile_pool(name="lg", bufs=2))

        # ---------------- phase 0: combined weights (PE) ----------------
        psw_ctx = ExitStack()
        prep = psw_ctx.enter_context(tc.tile_pool(name="prep", bufs=1))
        psw = psw_ctx.enter_context(tc.tile_pool(name="psw", bufs=2, space="PSUM"))
        rwt_t, cw1_t = [], []
        for k in range(2):
            rt = prep.tile([128, C], f32, tag=f"rwt{k}", name=f"rwt{k}")
            nc.sync.dma_start(rt[:], rwt_d.ap()[128 * k : 128 * (k + 1), :])
            rwt_t.append(rt)
            ct = prep.tile([128, MID], f32, tag=f"cw1{k}", name=f"cw1{k}")
            nc.sync.dma_start(ct[:], cw1_d.ap()[128 * k : 128 * (k + 1), :])
            cw1_t.append(ct)

        Wt = []
        for c in range(NCH):
            pw = psw.tile([128, 128], f32, tag="pw", name=f"pw{c}")
            for k in range(2):
                nc.tensor.matmul(
                    pw[:],
                    lhsT=rwt_t[k][:, 128 * c : 128 * (c + 1)],
                    rhs=cw1_t[k][:],
                    start=(k == 0),
                    stop=(k == 1),
                )
            wt = consts.tile([128, 128], bf16, tag=f"W{c}", name=f"W{c}")
            nc.scalar.copy(wt[:], pw[:])
            Wt.append(wt)

        rb_t = prep.tile([128, 2], f32, tag="rb", name="rb")
        nc.scalar.dma_start(rb_t[:], rb_d.ap().rearrange("(k p) -> p k", p=128))
        cb1_t = prep.tile([1, MID], f32, tag="cb1", name="cb1")
        nc.scalar.dma_start(cb1_t[:], cb1_d.ap().rearrange("(one d) -> one d", one=1))
        pb = psw.tile([1, 128], f32, tag="pb", name="pb")
        for k in range(2):
            nc.tensor.matmul(
                pb[:], lhsT=rb_t[:, k : k + 1], rhs=cw1_t[k][:],
                start=(k == 0), stop=(k == 1),
            )
        brow = prep.tile([1, 128], f32, tag="brow", name="brow")
        nc.vector.tensor_tensor(out=brow[:], in0=pb[:], in1=cb1_t[:], op=OP.add)
        bbias = consts.tile([1, 128], bf16, tag="bbias", name="bbias")
        nc.scalar.copy(bbias[:], brow[:])

        ones_t = consts.tile([1, PIX], bf16, tag="ones", name="ones")
        nc.vector.memset(ones_t[:], 1.0)
        ident = consts.tile([128, 128], bf16, tag="ident", name="ident")
        nc.scalar.dma_start(ident[:], id_d.ap())
        w2rf = prep.tile([128, 128], f32, tag="w2rf", name="w2rf")
        nc.scalar.dma_start(w2rf[:], w2r_d.ap())
        w2rep = consts.tile([128, 128], bf16, tag="w2rep", name="w2rep")
        nc.vector.tensor_copy(out=w2rep[:], in_=w2rf[:])
        cb2_t = consts.tile([128, 1], f32, tag="cb2", name="cb2")
        nc.scalar.dma_start(cb2_t[:], cb2_d.ap())
        w2big = consts.tile([128, VROW * 128], bf16, tag="w2big", name="w2big")
        for r9 in range(VROW):
            nc.vector.tensor_copy(out=w2big[:, 128 * r9 : 128 * (r9 + 1)], in_=w2rf[:])
        psw_ctx.close()

        zps = ctx.enter_context(tc.tile_pool(name="zps", bufs=2, space="PSUM"))
        pst = ctx.enter_context(tc.tile_pool(name="pst", bufs=3, space="PSUM"))

        def emit_floor(dst, srcap, nm, cols, pool):
            """dst = floor(srcap), srcap in [0, 32); robust to convert rounding."""
            ti = pool.tile([128, cols], i16, tag=f"flt_i{cols}", name=f"fi_{nm}")
            tf = pool.tile([128, cols], f32, tag=f"flt_f{cols}", name=f"ff_{nm}")
            nc.vector.tensor_copy(out=ti[:], in_=srcap)
            nc.vector.tensor_copy(out=dst, in_=ti[:])
            nc.vector.tensor_tensor(out=tf[:], in0=dst, in1=srcap, op=OP.is_gt)
            nc.vector.tensor_tensor(out=dst, in0=dst, in1=tf[:], op=OP.subtract)

        idx16_l, wx_l, wy_l = [], [], []
        for i in range(IMGS):
            # ---------------- wrapped idx prep for dma_gather ----------------
            # gather order g = j; idx for vert j at (partition j%16 (replicated
            # every 16), col j//16), int16.  uvw is host-pre-wrapped+replicated.
            uvwt = wp.tile([128, 2 * SW], f32, tag="uvwt", name=f"uvwt{i}")
            nc.scalar.dma_start(uvwt[:], uvw_d.ap()[i])
            pxw = wp.tile([128, SW], f32, tag="pxw", name=f"pxw{i}")
            pyw = wp.tile([128, SW], f32, tag="pyw", name=f"pyw{i}")
            nc.vector.tensor_scalar(out=pxw[:], in0=uvwt[:, 0 : 2 * SW : 2],
                                    scalar1=15.5, scalar2=15.5, op0=OP.mult, op1=OP.add)
            nc.vector.tensor_scalar(out=pyw[:], in0=uvwt[:, 1 : 2 * SW : 2],
                                    scalar1=15.5, scalar2=15.5, op0=OP.mult, op1=OP.add)
            x0w = wp.tile([128, SW], f32, tag="x0w", name=f"x0w{i}")
            y0w = wp.tile([128, SW], f32, tag="y0w", name=f"y0w{i}")
            emit_floor(x0w[:], pxw[:], f"xw{i}", SW, wp)
            emit_floor(y0w[:], pyw[:], f"yw{i}", SW, wp)
            nc.vector.tensor_scalar(out=x0w[:], in0=x0w[:], scalar1=30.0, scalar2=0.0,
                                    op0=OP.min, op1=OP.max)
            nc.vector.tensor_scalar(out=y0w[:], in0=y0w[:], scalar1=30.0, scalar2=0.0,
                                    op0=OP.min, op1=OP.max)
            idxwf = wp.tile([128, SW], f32, tag="pxw", name=f"idxwf{i}")
            nc.vector.scalar_tensor_tensor(
                out=idxwf[:], in0=y0w[:], scalar=32.0, in1=x0w[:],
                op0=OP.mult, op1=OP.add,
            )
            idx16 = idxp.tile([128, SW], i16, tag="idx16", name=f"idx16_{i}")
            nc.vector.tensor_copy(out=idx16[:], in_=idxwf[:])
            idx16_l.append(idx16)

            # ---------------- uv prep: blend weights (col-major layout) ----
            # vert j at (partition j%128, col j//128)
            uvt = sm.tile([128, 2 * Q], f32, tag="uvt", name=f"uvt{i}")
            uv_i = uv_d.ap()[i]
            nc.scalar.dma_start(
                uvt[:],
                AP(uv_i.tensor, uv_i.offset, [[2, 128], [256, Q], [1, 2]]),
            )
            px = sm.tile([128, Q], f32, tag="px", name=f"px{i}")
            py = sm.tile([128, Q], f32, tag="py", name=f"py{i}")
            nc.vector.tensor_scalar(out=px[:], in0=uvt[:, 0 : 2 * Q : 2],
                                    scalar1=15.5, scalar2=15.5, op0=OP.mult, op1=OP.add)
            nc.vector.tensor_scalar(out=py[:], in0=uvt[:, 1 : 2 * Q : 2],
                                    scalar1=15.5, scalar2=15.5, op0=OP.mult, op1=OP.add)
            x0 = sm.tile([128, Q], f32, tag="x0", name=f"x0{i}")
            y0 = sm.tile([128, Q], f32, tag="y0", name=f"y0{i}")
            emit_floor(x0[:], px[:], f"x{i}", Q, sm)
            emit_floor(y0[:], py[:], f"y{i}", Q, sm)
            nc.vector.tensor_scalar(out=x0[:], in0=x0[:], scalar1=30.0, scalar2=0.0,
                                    op0=OP.min, op1=OP.max)
            nc.vector.tensor_scalar(out=y0[:], in0=y0[:], scalar1=30.0, scalar2=0.0,
                                    op0=OP.min, op1=OP.max)
            wxf = sm.tile([128, Q], f32, tag="wxf", name=f"wxf{i}")
            wyf = sm.tile([128, Q], f32, tag="wyf", name=f"wyf{i}")
            nc.vector.tensor_tensor(out=wxf[:], in0=px[:], in1=x0[:], op=OP.subtract)
            nc.vector.tensor_tensor(out=wyf[:], in0=py[:], in1=y0[:], op=OP.subtract)
            wx = irp.tile([128, Q], bf16, tag="wx", name=f"wx{i}")
            wy = irp.tile([128, Q], bf16, tag="wy", name=f"wy{i}")
            nc.vector.tensor_copy(out=wx[:], in_=wxf[:])
            nc.vector.tensor_copy(out=wy[:], in_=wyf[:])
            wx_l.append(wx)
            wy_l.append(wy)

            # ---------------- z at pixels (PE) ----------------
            # feat loaded in two 5-channel-chunk halves to halve SBUF residency
            zp = zps.tile([128, PIX], f32, tag="zp", name=f"zp{i}")
            f_i = feat_d.ap()[i]
            NH = NCH // 2
            for h in range(2):
                ft = featp.tile([128, NH * PIX], bf16, tag="ft", name=f"ft{i}_{h}")
                nc.sync.dma_start(
                    ft[:],
                    AP(f_i.tensor, f_i.offset + h * NH * 128 * PIX,
                       [[PIX, 128], [128 * PIX, NH], [1, PIX]]),
                )
                for ph in range(2):
                    sl = slice(512 * ph, 512 * (ph + 1))
                    for c5 in range(NH):
                        nc.tensor.matmul(
                            zp[:, sl],
                            lhsT=Wt[NH * h + c5][:],
                            rhs=ft[:, PIX * c5 + 512 * ph : PIX * c5 + 512 * (ph + 1)],
                            start=(h == 0 and c5 == 0),
                            stop=False,
                            skip_group_check=True,
                        )
                    if h == 1:
                        nc.tensor.matmul(
                            zp[:, sl], lhsT=bbias[:], rhs=ones_t[:, sl],
                            start=False, stop=True, skip_group_check=True,
                        )

            # escape + pre-differenced quantities (dims-major, bf16)
            zq = zqp.tile([128, PPAD], bf16, tag="zq", name=f"zq{i}")
            dzx = zqp.tile([128, PPAD], bf16, tag="zq", name=f"dzx{i}")
            dzy = zqp.tile([128, PPAD], bf16, tag="zq", name=f"dzy{i}")
            dzxy = zqp.tile([128, PPAD], bf16, tag="zq", name=f"dzxy{i}")
            nc.scalar.copy(zq[:, 0:PIX], zp[:])
            nc.vector.memset(zq[:, PIX:PPAD], 0.0)
            nc.vector.tensor_tensor(out=dzx[:, 0:1056], in0=zq[:, 1:1057],
                                    in1=zq[:, 0:1056], op=OP.subtract)
            nc.vector.memset(dzx[:, 1056:PPAD], 0.0)
            nc.vector.tensor_tensor(out=dzy[:, 0:1056], in0=zq[:, 32:PPAD],
                                    in1=zq[:, 0:1056], op=OP.subtract)
            nc.vector.memset(dzy[:, 1056:PPAD], 0.0)
            nc.vector.tensor_tensor(out=dzxy[:, 0:1055], in0=dzy[:, 1:1056],
                                    in1=dzy[:, 0:1055], op=OP.subtract)
            nc.vector.memset(dzxy[:, 1055:PPAD], 0.0)

            # ---------------- tokens to DRAM (PE transpose per 128-pix block) ----
            stg = featp.tile([128, 8 * TOK], bf16, tag="stg", name=f"stg{i}")
            for b in range(8):
                pt = pst.tile([128, TOK], bf16, tag="pt", name=f"pt{i}_{b}")
                for qi, zt in enumerate((zq, dzx, dzy, dzxy)):
                    nc.tensor.transpose(
                        pt[:, 128 * qi : 128 * (qi + 1)],
                        zt[:, 128 * b : 128 * (b + 1)],
                        ident[:],
                    )
                nc.scalar.copy(stg[:, TOK * b : TOK * (b + 1)], pt[:])
            zt_i = ztok_d[i].ap()
            nc.sync.dma_start(
                AP(zt_i.tensor, zt_i.offset,
                   [[TOK, 128], [128 * TOK, 8], [1, TOK]]),
                stg[:].rearrange("p (b t) -> p b t", t=TOK),
            )


        for i in range(IMGS):
            # ---------------- gather + blend + dot per 2304-vert chunk ----------
            logit = lg.tile([128, Q], f32, tag="logit", name=f"lg{i}")
            NIC = GH // 16          # idx cols per gather
            for gk in range(3):
                gt = gpool.tile([128, GR * TOK], bf16, tag="g", name=f"g{i}_{gk}")
                g3full = gt[:].rearrange("p (r t) -> p r t", t=TOK)
                nc.gpsimd.dma_gather(
                    out_ap=g3full,
                    in_ap=ztok_d[i].ap(),
                    idxs_ap=idx16_l[i][:, NIC * gk : NIC * (gk + 1)],
                    num_idxs=GH,
                    num_idxs_reg=GH,
                    elem_size=TOK,
                    single_packet=False,
                )

                for sub in range(2):
                    ck = 2 * gk + sub
                    g3 = gt[:].rearrange("p (r t) -> p r t", t=TOK)[
                        :, VROW * sub : VROW * (sub + 1), :
                    ]

                    def wap(wtile, ck=ck):
                        a = wtile[:]
                        return AP(
                            a.tensor,
                            a.offset + VROW * ck * a.ap[-1][0],
                            [[a.ap[0][0], 128], [a.ap[-1][0], VROW], [0, 128]],
                        )

                    t1 = tpool.tile([128, VROW * 128], bf16, tag="t1", name=f"t1_{i}_{ck}")
                    t13 = t1[:].rearrange("p (r d) -> p r d", d=128)
                    acc = tpool.tile([128, VROW * 128], bf16, tag="acc", name=f"ac{i}_{ck}")
                    acc3 = acc[:].rearrange("p (r d) -> p r d", d=128)
                    # t1 = wx*dzx ; acc = z00 + t1
                    nc.vector.tensor_tensor(out=t13, in0=g3[:, :, 128:256], in1=wap(wx_l[i]), op=OP.mult)
                    nc.vector.tensor_tensor(out=acc3, in0=g3[:, :, 0:128], in1=t13, op=OP.add)
                    # t1 = wx*dzxy ; t1 += dzy ; t1 *= wy ; acc += t1
                    nc.vector.tensor_tensor(out=t13, in0=g3[:, :, 384:512], in1=wap(wx_l[i]), op=OP.mult)
                    nc.vector.tensor_tensor(out=t13, in0=g3[:, :, 256:384], in1=t13, op=OP.add)
                    nc.vector.tensor_tensor(out=t13, in0=t13, in1=wap(wy_l[i]), op=OP.mult)
                    nc.vector.tensor_tensor(out=acc3, in0=acc3, in1=t13, op=OP.add)
                    # h = relu(acc) on ACT, then flat contiguous w2 multiply (2x DVE)
                    rl = tpool.tile([128, VROW * 128], bf16, tag="rl", name=f"rl_{i}_{ck}")
                    nc.scalar.activation(rl[:], acc[:], ACT.Relu)
                    nc.vector.tensor_tensor(out=acc[:], in0=rl[:], in1=w2big[:], op=OP.mult)
                    nc.vector.tensor_reduce(
                        out=logit[:, VROW * ck : VROW * (ck + 1)].rearrange(
                            "p (r one) -> p r one", one=1
                        ),
                        in_=acc3,
                        axis=mybir.AxisListType.X,
                        op=OP.add,
                    )
            ostg = lg.tile([128, Q], f32, tag="ostg", name=f"os{i}")
            nc.scalar.activation(ostg[:], logit[:], ACT.Sigmoid, bias=cb2_t[:])
            o_i = out_d.ap()[i]
            oap = AP(o_i.tensor, o_i.offset, [[1, 128], [128, Q]])
            nc.scalar.dma_start(oap, ostg[:])

    nc.compile()
    _CACHE["nc"] = nc
    return nc


def _host_prep(inputs):
    feat = np.asarray(inputs["feat_map"], dtype=np.float32)
    uv = np.asarray(inputs["verts_uv"], dtype=np.float32)
    rw = np.asarray(inputs["reduce_w"], dtype=np.float32)
    rb = np.asarray(inputs["reduce_b"], dtype=np.float32)
    w1 = np.asarray(inputs["cls_w1"], dtype=np.float32)
    b1 = np.asarray(inputs["cls_b1"], dtype=np.float32)
    w2 = np.asarray(inputs["cls_w2"], dtype=np.float32)
    b2 = np.asarray(inputs["cls_b2"], dtype=np.float32)

    rwt = np.ascontiguousarray(rw.T)                      # (256, 1280)
    uvp = np.zeros((B, NV, 2), dtype=np.float32)
    uvp[:, :N, :] = uv
    # wrapped copy for the gather-index prep: vert j at (partition j%16
    # replicated every 16, col j//16); padded to 8192 verts.
    uvw = np.zeros((B, NVW, 2), dtype=np.float32)
    uvw[:, :N, :] = uv
    uvw = uvw.reshape(B, SW, 16, 2).transpose(0, 2, 1, 3)   # (B,16,512,2)
    uvw = np.tile(uvw, (1, 8, 1, 1)).reshape(B, 128, 2 * SW)
    featr = feat.reshape(B, C, PIX).astype(ml_dtypes.bfloat16)

    shared = {
        "rwt": rwt,
        "cw1": np.ascontiguousarray(w1),
        "rb": rb,
        "cb1": b1,
        "w2r": np.ascontiguousarray(np.tile(w2[None, :], (128, 1))),
        "cb2": np.full((128, 1), b2[0], dtype=np.float32),
        "ident": np.eye(128, dtype=ml_dtypes.bfloat16),
    }
    in_maps = []
    for core in range(NCORES):
        sl = slice(core * IMGS, (core + 1) * IMGS)
        m = dict(shared)
        m["feat"] = np.ascontiguousarray(featr[sl])
        m["uv"] = np.ascontiguousarray(uvp[sl])
        m["uvw"] = np.ascontiguousarray(uvw[sl])
        in_maps.append(m)
    return in_maps


def kernel(**inputs):
    from concourse.bass_utils import run_bass_kernel_spmd

    nc = _build()
    in_maps = _host_prep(inputs)
    res = run_bass_kernel_spmd(nc, in_maps, list(range(NCORES)))
    out = np.empty((B, N), dtype=np.float32)
    for core in range(NCORES):
        dev = res.results[core]["out"]          # (IMGS, NV), vert j at col j
        out[core * IMGS : (core + 1) * IMGS] = dev[:, :N]
    return out
